# revision 4
# baseline (speedup 1.0000x reference)
"""Channel attention kernel for Trainium2, 8-core data parallel.

Computes, per batch b:
    X   = x[b].reshape(C, H*W)            # (512, 2304)
    G   = X @ X.T                         # (512, 512) Gram
    A   = softmax(G, axis=1)
    agg = A @ X                           # (512, 2304)
    out[b] = x[b] + scale * agg

Sharding: pure data parallel over the batch dim n=64 -> 8 batches per core.

Per-core pipeline (fp8e4 matmul operands with DoubleRow perf mode: the PE
contracts 256 per instruction and streams ~2 output cols/cycle; fp32
accumulation in PSUM; the softmax itself runs in fp32, and the enormous
Gram-diagonal margin (~2000 logits) makes A == I to fp32 precision, so
fp8 operand rounding does not perturb the attention weights):
  1. DMA x[b] into 4 SBUF tiles X[cb]=[128,2304] f32 (column-split DMAs).
     X stays exact fp32 for the residual.
  2. ACT casts X -> xq8 "pair" tiles [128, 2, 2304] fp8 (partition p,
     group i holds channel pair_base + i*128 + p) - mm2's moving operand
     and the source for the X transposes.
  3. PE-transpose xq8 -> xtq8 (9 pair tiles [128d, 2, 512c] fp8) via fp8
     transposes (1 cyc/row) against a bf16 identity; 4 transposes of one
     kb share one fp8 PSUM bank, evacuated by a single ACT copy.
  4. mm1 (DoubleRow): G (PSUM f32) += xtq8[j][:, :, mb]^T @ xtq8[j],
     j = 0..8, one G bank per mb block.
  5. softmax: row max (DVE, negated) -> exp with bias + fused row-sum
     (ACT accum_out) writing E directly as fp8 [128,512]; reciprocal;
     normalization deferred into the final residual scale.
  6. PE-transpose E per mb into a shared fp8 PSUM bank; 2 ACT copies
     scatter it into the etT8 pair tiles [128k, 2, 512c].
  7. mm2 (DoubleRow): Y (PSUM f32) += etT8[j][:, :, mb]^T @ xq8[j][:, :, chunk].
  8. out = (Y * (scale/rowsum)) + X in one DVE scalar_tensor_tensor,
     then DMA out.
"""

import numpy as np
from contextlib import ExitStack

import concourse.bass as bass
import concourse.bacc as bacc
import concourse.tile as tile
from concourse import mybir
from concourse.masks import make_identity
from concourse.bass_utils import run_bass_kernel_spmd

N_CORES = 8
N, C, H, W = 64, 512, 48, 48
HW = H * W                    # 2304
B = N // N_CORES              # 8 batches per core
P = 128
NCB = C // P                  # 4 c-blocks
NDB = HW // P                 # 18 d-blocks
NPAIR = NDB // 2              # 9 d-block pairs (DoubleRow contraction)
F32 = mybir.dt.float32
BF16 = mybir.dt.bfloat16
F8 = mybir.dt.float8e4
DR = mybir.MatmulPerfMode.DoubleRow

# d-chunks for mm2 / residual / store: 4 x 512 + 1 x 256
CHUNKS = [(i * 512, min(512, HW - i * 512)) for i in range((HW + 511) // 512)]
# column pieces for load/cast pipelining
PIECES = [(0, 512), (512, 1408), (1408, HW)]

AX = mybir.AxisListType.X
MULT = mybir.AluOpType.mult
ADD = mybir.AluOpType.add
EXP = mybir.ActivationFunctionType.Exp
COPY = mybir.ActivationFunctionType.Copy

N_WARM = 44


def _build():
    nc = bacc.Bacc()
    x_d = nc.dram_tensor("x", [B, C, HW], F32, kind="ExternalInput")
    s_d = nc.dram_tensor("scale", [1], F32, kind="ExternalInput")
    o_d = nc.dram_tensor("out", [B, C, HW], F32, kind="ExternalOutput")

    with tile.TileContext(nc) as tc:
        with ExitStack() as ctx:
            singles = ctx.enter_context(tc.tile_pool(name="singles", bufs=1))
            xpool = ctx.enter_context(tc.tile_pool(name="xp", bufs=7))
            xqpool = ctx.enter_context(tc.tile_pool(name="xqp", bufs=5))
            xtpool = ctx.enter_context(tc.tile_pool(name="xtp", bufs=19))
            epool = ctx.enter_context(tc.tile_pool(name="ep", bufs=5))
            etpool = ctx.enter_context(tc.tile_pool(name="etp", bufs=5))
            opool = ctx.enter_context(tc.tile_pool(name="op", bufs=5))
            stats = ctx.enter_context(tc.tile_pool(name="st", bufs=24))
            tpsum = ctx.enter_context(
                tc.tile_pool(name="tps", bufs=2, space="PSUM"))
            etpsum = ctx.enter_context(
                tc.tile_pool(name="etps", bufs=2, space="PSUM"))
            gpsum = ctx.enter_context(
                tc.tile_pool(name="gps", bufs=2, space="PSUM"))
            ypsum = ctx.enter_context(
                tc.tile_pool(name="yps", bufs=2, space="PSUM"))

            id_f32 = singles.tile([P, P], F32)
            make_identity(nc, id_f32[:])
            id_f8 = singles.tile([P, P], F8)
            nc.gpsimd.tensor_copy(out=id_f8[:], in_=id_f32[:])
            scale_sb = singles.tile([P, 1], F32)
            nc.sync.dma_start(out=scale_sb[:], in_=s_d.broadcast_to([P, 1]))

            # Dummy transposes: let the PE observe the gpsimd-produced
            # identities once here so real matmuls never need that wait
            # (matmuls have a single sync-wait slot in walrus codegen),
            # and keep the PE busy ~3.5us so batch 0 runs at 2.4 GHz.
            def s2(ps, blk):
                # stride-2 fp8 PSUM view of 128-col transpose block `blk`
                return ps[:].rearrange("p (c two) -> p c two", two=2)[
                    :, blk * P:(blk + 1) * P, 0]

            for w in range(N_WARM):
                wt = tpsum.tile([P, 2 * C], F8, tag="tps", name=f"warm{w}")
                nc.tensor.transpose(s2(wt, 0), id_f8[:], id_f8[:])

            for b in range(B):
                # ---- load X (natural layout, 4 tiles of [128, 2304]) ----
                xs = []
                for cb in range(NCB):
                    xt = xpool.tile([P, HW], F32, tag="x", name=f"x{cb}")
                    xs.append(xt)
                # fp8 pair tiles: xq8[j][p, i, :] = channels j*256 + i*128 + p
                xq8 = [xqpool.tile([P, 2, HW], F8, tag="xq", name=f"xq{j}")
                       for j in range(2)]
                # First 512 columns land first so casts/transposes start
                # early; the rest follows on other queues.
                for p0, p1 in PIECES:
                    for cb in range(NCB):
                        nc.sync.dma_start(
                            out=xs[cb][:, p0:p1],
                            in_=x_d[b, cb * P:(cb + 1) * P, p0:p1])

                # ---- ACT: cast X -> fp8 pair tiles, piece by piece ----
                def emit_casts(pi):
                    p0, p1 = PIECES[pi]
                    for cb in range(NCB):
                        nc.scalar.copy(
                            out=xq8[cb // 2][:, cb % 2, p0:p1],
                            in_=xs[cb][:, p0:p1])

                emit_casts(0)

                # ---- transpose xq8 -> xtq8 (PE, fp8) ----
                # xtq8[j][p, i, c] = X[c, d = j*256 + i*128 + p]
                xtq8 = [xtpool.tile([P, 2, C], F8, tag="xt", name=f"xT{j}")
                        for j in range(NPAIR)]
                for kb in range(NDB):
                    if kb == 4:
                        emit_casts(1)
                    elif kb == 11:
                        emit_casts(2)
                    ps = tpsum.tile([P, 2 * C], F8, tag="tps")
                    for cb in range(NCB):
                        nc.tensor.transpose(
                            s2(ps, cb),
                            xq8[cb // 2][:, cb % 2, kb * P:(kb + 1) * P],
                            id_f8[:])
                    nc.scalar.copy(
                        out=xtq8[kb // 2][:, kb % 2, :],
                        in_=ps[:].rearrange(
                            "p (c two) -> p c two", two=2)[:, :, 0])

                # ---- mm1 (DoubleRow) + softmax + E transpose ----
                alphas = []
                es = []

                def emit_etrans(m):
                    ps = etpsum.tile([P, 2 * C], F8, tag="et", name=f"eps{m}")
                    for kb in range(NCB):
                        nc.tensor.transpose(
                            s2(ps, kb),
                            es[m][:, kb * P:(kb + 1) * P], id_f8[:])
                    for kb in range(NCB):
                        nc.scalar.copy(
                            out=etT8[kb // 2][:, kb % 2, m * P:(m + 1) * P],
                            in_=s2(ps, kb))

                etT8 = [etpool.tile([P, 2, C], F8, tag="ett", name=f"eT{j}")
                        for j in range(2)]
                for mb in range(NCB):
                    G = gpsum.tile([P, C], F32, tag="g", name=f"G{mb}")
                    for j in range(NPAIR):
                        nc.tensor.matmul(
                            G[:],
                            xtq8[j][:, :, mb * P:(mb + 1) * P],
                            xtq8[j][:],
                            start=(j == 0), stop=(j == NPAIR - 1),
                            perf_mode=DR)
                    neg_m = stats.tile([P, 1], F32, tag="negm")
                    nc.vector.reduce_max(
                        out=neg_m[:], in_=G[:], axis=AX, negate=True)
                    e = epool.tile([P, C], F8, tag="e")
                    s = stats.tile([P, 1], F32, tag="s")
                    nc.scalar.activation(
                        out=e[:], in_=G[:], func=EXP,
                        bias=neg_m[:], scale=1.0, accum_out=s[:])
                    rs = stats.tile([P, 1], F32, tag="rs")
                    nc.vector.reciprocal(out=rs[:], in_=s[:])
                    alpha = stats.tile([P, 1], F32, tag="al")
                    nc.vector.tensor_mul(alpha[:], rs[:], scale_sb[:])
                    alphas.append(alpha)
                    es.append(e)
                    # E-transpose for the PREVIOUS block, emitted inside
                    # the mm1 loop so its ACT copies hide behind mm1.
                    if mb >= 1:
                        emit_etrans(mb - 1)
                emit_etrans(NCB - 1)

                # ---- mm2 (DoubleRow) + fused residual + store ----
                for mb in range(NCB):
                    for ci, (c0, csz) in enumerate(CHUNKS):
                        y = ypsum.tile([P, 512], F32, tag="y")
                        for j in range(2):
                            nc.tensor.matmul(
                                y[:, :csz],
                                etT8[j][:, :, mb * P:(mb + 1) * P],
                                xq8[j][:, :, c0:c0 + csz],
                                start=(j == 0), stop=(j == 1),
                                perf_mode=DR)
                        o = opool.tile([P, 512], F32, tag="o")
                        nc.vector.scalar_tensor_tensor(
                            out=o[:, :csz], in0=y[:, :csz],
                            scalar=alphas[mb][:],
                            in1=xs[mb][:, c0:c0 + csz],
                            op0=MULT, op1=ADD)
                        nc.sync.dma_start(
                            out=o_d[b, mb * P:(mb + 1) * P, c0:c0 + csz],
                            in_=o[:, :csz])
    nc.finalize()
    return nc


def _ensure_ntff_hook():
    """Install the axon NTFF profiling hook if the image's antenv lacks it.

    Only needed for trace=True runs (local perf iteration); the grading
    path never calls this.
    """
    import sys
    import types
    try:
        from antenv import axon_hooks  # noqa: F401
        return
    except ImportError:
        pass
    mod = types.ModuleType("antenv.axon_hooks")
    _h = {"hook": None}
    mod.set_axon_ntff_profile_hook = lambda h: _h.__setitem__("hook", h)
    mod.get_axon_ntff_profile_hook = lambda: _h["hook"]
    sys.modules["antenv.axon_hooks"] = mod
    import antenv
    antenv.axon_hooks = mod
    try:
        from trn_agent_boot.trn_boot import _ntff_profile_via_ctypes
        hook = _ntff_profile_via_ctypes("/opt/axon/libaxon_pjrt.so")
        if hook is not None:
            mod.set_axon_ntff_profile_hook(hook)
    except Exception:
        pass


_NC_CACHE = {}


def _get_nc(key=0):
    if key not in _NC_CACHE:
        _NC_CACHE[key] = _build()
    return _NC_CACHE[key]


def kernel(x, scale, trace=False, use_f32r=True):
    x = np.ascontiguousarray(x, dtype=np.float32)
    scale = np.ascontiguousarray(scale, dtype=np.float32)
    if trace:
        _ensure_ntff_hook()
    nc = _get_nc()
    xr = x.reshape(N, C, HW)
    in_maps = [
        {"x": xr[i * B:(i + 1) * B], "scale": scale}
        for i in range(N_CORES)
    ]
    res = run_bass_kernel_spmd(
        nc, in_maps, core_ids=list(range(N_CORES)), trace=trace)
    out = np.concatenate([r["out"] for r in res.results], axis=0)
    out = out.reshape(N, C, H, W)
    if trace:
        kernel.last_exec_time_ns = res.exec_time_ns
        kernel.last_results = res
    return out
